# revision 1
# baseline (speedup 1.0000x reference)
"""GAT (2-layer) Trainium2 kernel, SPMD across 8 NeuronCores.

Key algebra: segment softmax keyed by row is shift invariant, so the
(h[row] . a_l) term cancels and attention factorizes:
    alpha_e = g[col_e] * u[row_e],
    g[n] = exp(h[n] . a_r),   u[r] = 1 / sum_{e: row=r} g[col_e]
Each GAT layer then needs only two unweighted sparse ops over the fixed
graph:
    z   = A @ g          (segment-sum keyed by row)   -> u = 1/z
    agg = A^T @ (u * h)  (segment-sum keyed by col)
    out = g * agg
Both are done as: dma_gather of table rows per edge (128 edges/block) +
one-hot matmul (lhsT = one-hot of block-relative destination, built by a
DVE is_equal against an iota tile) accumulating into a PSUM window.

Sharding: z-phase edges by row range, aggregation edges by col range (each
core owns its 1250-node output slice). Cross-core: AllGather of u
([10000,H] f32) and of h1^T (5 MB) between the layers.

kernel(**inputs) takes FULL inputs and returns the FULL [10000, 22] output.
"""

import sys

sys.path.insert(0, "/opt/trn_rl_repo")

import numpy as np
import ml_dtypes

from concourse import bacc, mybir, tile
from concourse.bass_utils import run_bass_kernel_spmd

F32 = mybir.dt.float32
BF16 = mybir.dt.bfloat16
I16 = mybir.dt.int16
EXP = mybir.ActivationFunctionType.Exp
EQ = mybir.AluOpType.is_equal
MULT = mybir.AluOpType.mult
ADD = mybir.AluOpType.add
MIN = mybir.AluOpType.min
BYPASS = mybir.AluOpType.bypass

N = 10000
E = 320000
F = 128
H = 4
C = 22
P = 8
SLICE = N // P               # 1250 nodes per core
NWIN = (SLICE + 127) // 128  # 10 windows of <=128 dst/src nodes
NBLK = N // 128 + 1          # 79; always >= 1 pad block so row N is zero
NPAD = NBLK * 128            # 10112; table rows >= N are zero
OW1 = H * F                  # 512
CHUNK = 16                   # layer-1 gather chunk (128-edge blocks)
SKIP = set()                 # debug/timing: {"z", "agg1", "agg2"}


def _configure(n, e, p=8):
    """Shrink sizes for simulator debugging (same program structure)."""
    global N, E, P, SLICE, NWIN, NBLK, NPAD
    N, E, P = n, e, p
    SLICE = N // P
    NWIN = (SLICE + 127) // 128
    NBLK = N // 128 + 1
    NPAD = NBLK * 128


def _cdiv(a, b):
    return (a + b - 1) // b


def _wrap_idxs(idx):
    """dma_gather index layout: logical i at [i%16, i//16], replicated to
    128 partitions."""
    n = idx.shape[0]
    assert n % 16 == 0
    w = idx.reshape(n // 16, 16).T.astype(np.int16)
    return np.tile(w, (8, 1))


def _phase_arrays(key, other, nwin):
    """Group one core's (already core-local) edges by 128-wide key window.
    Returns per-window (rel, other) with rel = key - 128*w."""
    w = key >> 7
    order = np.argsort(w, kind="stable")
    key, other, w = key[order], other[order], w[order]
    out = []
    bounds = np.searchsorted(w, np.arange(nwin + 1))
    for i in range(nwin):
        sl = slice(bounds[i], bounds[i + 1])
        k, o = key[sl] - 128 * i, other[sl]
        so = np.argsort(o, kind="stable")  # sorted gather idx -> HBM locality
        out.append((k[so], o[so]))
    return out


def _build_edge_inputs(row, col):
    zraw, braw = [], []
    for k in range(P):
        base = k * SLICE
        m = (row >= base) & (row < base + SLICE)
        zraw.append(_phase_arrays(row[m] - base, col[m], NWIN))
        m = (col >= base) & (col < base + SLICE)
        braw.append(_phase_arrays(col[m] - base, row[m], NWIN))

    def block_counts(raw):
        return [
            max(_cdiv(max(max(len(raw[k][w][0]) for k in range(P)), 1), 128), 1)
            for w in range(NWIN)
        ]

    zB = block_counts(zraw)
    bB = block_counts(braw)

    def pack(raw, B):
        idx_l, rel_l = [], []
        for w in range(NWIN):
            n = B[w] * 128
            rel = np.zeros(n, np.int32)
            oth = np.full(n, N, np.int32)  # dummy -> zero table row
            r, o = raw[w]
            rel[: len(r)] = r
            oth[: len(o)] = o
            idx_l.append(_wrap_idxs(oth))
            rel_l.append(rel.reshape(B[w], 128).T.astype(np.float32))
        return np.concatenate(idx_l, 1), np.concatenate(rel_l, 1)

    per_core = []
    for k in range(P):
        zidx, zrel = pack(zraw[k], zB)
        bidx, brel = pack(braw[k], bB)
        base = k * SLICE
        gw = []
        for w in range(NWIN):
            nid = base + 128 * w + np.arange(128)
            nid = np.where(nid < base + SLICE, nid, N)
            gw.append(_wrap_idxs(nid))
        per_core.append(
            dict(
                zidx=zidx,
                zrel=zrel,
                bidx=bidx,
                brel_f=brel,
                gwidx=np.concatenate(gw, 1),
            )
        )
    return zB, bB, per_core


def _spmm(nc, tc, B, CH, idx_d, rel_d, tab, elem, rhs_w, psum_w, iof_t,
          name, flush, skip=False, bufs=3):
    """One-hot-matmul SpMM over 128-dst windows with gather chunks that span
    window boundaries. flush(w, po) consumes each window's PSUM result."""
    with (
        tc.tile_pool(name=f"gg{name}", bufs=bufs) as ggp,
        tc.tile_pool(name=f"gi{name}", bufs=bufs) as gip,
        tc.tile_pool(name=f"gr{name}", bufs=bufs) as grp,
        tc.tile_pool(name=f"go{name}", bufs=bufs) as ohp,
        tc.tile_pool(name=f"gp{name}", bufs=2, space="PSUM") as pp,
    ):
        total = sum(B)
        gts, ohs = {}, {}
        gb = 0
        for w, Bw in enumerate(B):
            po = pp.tile([128, psum_w], F32, tag="po")
            if skip:
                nc.vector.memset(po[:], 1.0)
                flush(w, po)
                continue
            for b in range(Bw):
                ch, off = divmod(gb, CH)
                if off == 0:
                    cb = min(CH, total - ch * CH)
                    it = gip.tile([128, CH * 8], I16, tag="gi")
                    nc.sync.dma_start(
                        it[:, : cb * 8],
                        idx_d[:, ch * CH * 8 : (ch * CH + cb) * 8],
                    )
                    gt = ggp.tile([128, CH, elem], BF16, tag="gg")
                    nc.gpsimd.dma_gather(
                        gt[:, :cb, :], tab[:], it[:, : cb * 8],
                        cb * 128, cb * 128, elem, single_packet=False,
                    )
                    rl = grp.tile([128, CH], F32, tag="gr")
                    nc.sync.dma_start(
                        rl[:, :cb], rel_d[:, ch * CH : ch * CH + cb]
                    )
                    oh = ohp.tile([128, CH, 128], BF16, tag="go")
                    nc.vector.tensor_tensor(
                        oh[:, :cb, :],
                        iof_t[:].rearrange("p (x f) -> p x f", x=1)
                        .broadcast_to([128, cb, 128]),
                        rl[:, :cb].rearrange("p (b x) -> p b x", x=1)
                        .broadcast_to([128, cb, 128]),
                        EQ,
                    )
                    gts[ch], ohs[ch] = gt, oh
                nc.tensor.matmul(
                    po[:], ohs[ch][:, off, :], gts[ch][:, off, 0:rhs_w],
                    start=(b == 0), stop=(b == Bw - 1),
                )
                gb += 1
            flush(w, po)


def _declare(nc, zB, bB):
    ZT, BT = sum(zB), sum(bB)
    T = type("T", (), {})()
    T.xT = nc.dram_tensor("xT", [F, NPAD], F32, kind="ExternalInput")
    T.W1 = nc.dram_tensor("W1", [F, OW1], F32, kind="ExternalInput")
    T.W2 = nc.dram_tensor("W2", [F, C], F32, kind="ExternalInput")
    T.a1rc = nc.dram_tensor("a1rc", [F, H], F32, kind="ExternalInput")
    T.a2rc = nc.dram_tensor("a2rc", [F, 1], F32, kind="ExternalInput")
    T.ident = nc.dram_tensor("ident", [128, 128], F32, kind="ExternalInput")
    T.iota_f = nc.dram_tensor("iota_f", [128, 128], F32, kind="ExternalInput")
    T.zidx_d = nc.dram_tensor("zidx", [128, ZT * 8], I16, kind="ExternalInput")
    T.zrel_d = nc.dram_tensor("zrel", [128, ZT], F32, kind="ExternalInput")
    T.bidx_d = nc.dram_tensor("bidx", [128, BT * 8], I16, kind="ExternalInput")
    T.brelf_d = nc.dram_tensor("brel_f", [128, BT], F32, kind="ExternalInput")
    T.gwidx_d = nc.dram_tensor("gwidx", [128, NWIN * 8], I16, kind="ExternalInput")
    T.out_d = nc.dram_tensor("out", [SLICE, C], F32, kind="ExternalOutput")

    T.g1_tab = nc.dram_tensor("g1_tab", [NPAD, 128], BF16)
    T.hh1_tab = nc.dram_tensor("hh1_tab", [NPAD, OW1], BF16)
    T.g2_tab = nc.dram_tensor("g2_tab", [NPAD, 128], BF16)
    T.hh2_tab = nc.dram_tensor("hh2_tab", [NPAD, 128], BF16)
    T.u1_sl = nc.dram_tensor("u1_sl", [SLICE, H], F32)
    T.u2_sl = nc.dram_tensor("u2_sl", [SLICE, 1], F32)
    T.u1_full = nc.dram_tensor("u1_full", [NPAD, H], F32, addr_space="Shared")
    T.u2_full = nc.dram_tensor("u2_full", [NPAD, 1], F32, addr_space="Shared")
    T.h1T_loc = nc.dram_tensor("h1T_loc", [F, SLICE], F32)
    T.h1T_ag = nc.dram_tensor("h1T_ag", [P, F, SLICE], F32, addr_space="Shared")

    return T


def _emit(nc, tc, T, zB, bB, s=""):
        groups = [list(range(P))]
        # ================= layer 1: dense + tables + z1 =================
        with (
            tc.tile_pool(name="persist" + s, bufs=1) as pp,
            tc.tile_pool(name="small" + s, bufs=3) as sp,
        ):
            W1_t = pp.tile([F, OW1], F32)
            nc.sync.dma_start(W1_t[:], T.W1[:])
            id_t = pp.tile([128, 128], F32)
            nc.sync.dma_start(id_t[:], T.ident[:])
            iof_t = pp.tile([128, 128], F32)
            nc.sync.dma_start(iof_t[:], T.iota_f[:])
            a1rc_t = pp.tile([F, H], F32)
            nc.sync.dma_start(a1rc_t[:], T.a1rc[:])
            W1ar_t = pp.tile([F, H], F32)

            with tc.tile_pool(name="ptr" + s, bufs=2, space="PSUM") as ptr:
                for hd in range(H):
                    pt = ptr.tile([128, 128], F32, tag="pt")
                    nc.tensor.transpose(pt[:], W1_t[:, hd * F : (hd + 1) * F], id_t[:])
                    w1t = sp.tile([128, 128], F32, tag="w1t")
                    nc.vector.tensor_copy(w1t[:], pt[:])
                    pv = ptr.tile([128, 1], F32, tag="pv")
                    nc.tensor.matmul(
                        pv[:], w1t[:], a1rc_t[:, hd : hd + 1], start=True, stop=True
                    )
                    nc.vector.tensor_copy(W1ar_t[:, hd : hd + 1], pv[:])

            h_nm = pp.tile([128, NBLK, OW1], F32)  # 20.2 MB
            g1_nm = pp.tile([128, NBLK, H], F32)
            with (
                tc.tile_pool(name="xtp" + s, bufs=3) as xtp,
                tc.tile_pool(name="ph" + s, bufs=2, space="PSUM") as php,
                tc.tile_pool(name="psr" + s, bufs=2, space="PSUM") as psrp,
            ):
                for b in range(NBLK):
                    xt = xtp.tile([128, 128], F32)
                    nc.sync.dma_start(xt[:], T.xT[:, b * 128 : (b + 1) * 128])
                    ph = php.tile([128, OW1], F32)
                    nc.tensor.matmul(ph[:], xt[:], W1_t[:], start=True, stop=True)
                    psr = psrp.tile([128, H], F32)
                    nc.tensor.matmul(psr[:], xt[:], W1ar_t[:], start=True, stop=True)
                    nc.vector.tensor_copy(h_nm[:, b, :], ph[:])
                    nc.scalar.activation(g1_nm[:, b, :], psr[:], EXP)

            with tc.tile_pool(name="stage" + s, bufs=1) as stp:
                st = stp.tile([128, NBLK, 128], BF16, tag="stage")
                nc.vector.memset(st[:], 0.0)
                nc.vector.tensor_copy(
                    st[:, : NBLK - 1, 0:H], g1_nm[:, : NBLK - 1, :]
                )
                nv = N - 128 * (NBLK - 1)
                if nv > 0:
                    nc.vector.tensor_copy(
                        st[0:nv, NBLK - 1, 0:H], g1_nm[0:nv, NBLK - 1, :]
                    )
                nc.sync.dma_start(
                    T.g1_tab.ap().rearrange("(b p) c -> p b c", p=128), st[:]
                )

            with tc.tile_pool(name="zu1" + s, bufs=3) as zup:

                def zflush1(w, po, zup=zup):
                    u_t = zup.tile([128, H], F32, tag="u")
                    nc.vector.reciprocal(u_t[:], po[:, 0:H])
                    rows = min(128, SLICE - 128 * w)
                    nc.sync.dma_start(
                        T.u1_sl[w * 128 : w * 128 + rows, :], u_t[0:rows, :]
                    )

                _spmm(nc, tc, zB, 32, T.zidx_d, T.zrel_d, T.g1_tab, 128, 8, 8,
                      iof_t, "z1" + s, zflush1, skip=("z" in SKIP), bufs=2)

            nc.gpsimd.collective_compute(
                "AllGather", BYPASS, groups,
                ins=[T.u1_sl[:].opt()], outs=[T.u1_full[0:N, :].opt()],
            )
            zt = sp.tile([NPAD - N, H], F32, tag="zpad")
            nc.vector.memset(zt[:], 0.0)
            nc.sync.dma_start(T.u1_full[N:NPAD, :], zt[:])

            u1_nm = pp.tile([128, NBLK, H], F32)
            nc.sync.dma_start(
                u1_nm[:], T.u1_full.ap().rearrange("(b p) c -> p b c", p=128)
            )
            with tc.tile_pool(name="hhp" + s, bufs=3) as hhp:
                for b in range(NBLK):
                    hh = hhp.tile([128, OW1], BF16)
                    for hd in range(H):
                        nc.vector.tensor_scalar(
                            hh[:, hd * F : (hd + 1) * F],
                            h_nm[:, b, hd * F : (hd + 1) * F],
                            u1_nm[:, b, hd : hd + 1],
                            None,
                            MULT,
                        )
                    nc.sync.dma_start(
                        T.hh1_tab.ap().rearrange("(b p) c -> p b c", p=128)[:, b, :],
                        hh[:],
                    )

        # ============ layer 1 aggregation + layer 2 (h_nm freed) ============
        with (
            tc.tile_pool(name="persist2" + s, bufs=1) as pp2,
            tc.tile_pool(name="small2" + s, bufs=3) as sp2,
        ):
            iof2 = pp2.tile([128, 128], F32)
            nc.sync.dma_start(iof2[:], T.iota_f[:])
            id2 = pp2.tile([128, 128], F32)
            nc.sync.dma_start(id2[:], T.ident[:])
            W2cat = pp2.tile([F, C + 1], F32)
            nc.sync.dma_start(W2cat[:, 0:C], T.W2[:])
            with tc.tile_pool(name="ptr2" + s, bufs=2, space="PSUM") as ptr:
                a2rc_t = sp2.tile([F, 1], F32, tag="T.a2rc")
                nc.sync.dma_start(a2rc_t[:], T.a2rc[:])
                pt = ptr.tile([128, 128], F32, tag="pt2")
                nc.tensor.transpose(pt[0:C, :], W2cat[:, 0:C], id2[:])
                w2t = sp2.tile([128, 128], F32, tag="w2t")
                nc.vector.tensor_copy(w2t[0:C, :], pt[0:C, :])
                pv = ptr.tile([128, 1], F32, tag="pv2")
                nc.tensor.matmul(
                    pv[:], w2t[0:C, :], a2rc_t[0:C, :], start=True, stop=True
                )
                nc.vector.tensor_copy(W2cat[:, C : C + 1], pv[:])

            h1T_sb = pp2.tile([128, NWIN * 128], F32)

            with (
                tc.tile_pool(name="gwp" + s, bufs=2) as gwp,
                tc.tile_pool(name="ptw" + s, bufs=2, space="PSUM") as ptw,
                tc.tile_pool(name="flush" + s, bufs=2) as flp,
            ):
                gwi = gwp.tile([128, NWIN * 8], I16, tag="gwi")
                nc.sync.dma_start(gwi[:], T.gwidx_d[:])
                gwb = gwp.tile([128, NWIN, 128], BF16, tag="gwb")
                nc.gpsimd.dma_gather(
                    gwb[:], T.g1_tab[:], gwi[:], NWIN * 128, NWIN * 128, 128,
                    single_packet=False,
                )
                gwf = gwp.tile([128, NWIN, 128], F32, tag="gwf")
                nc.vector.tensor_copy(gwf[:], gwb[:])

                def flush1(w, po):
                    o_t = flp.tile([128, OW1], F32, tag="o")
                    for hd in range(H):
                        nc.vector.tensor_scalar(
                            o_t[:, hd * F : (hd + 1) * F],
                            po[:, hd * F : (hd + 1) * F],
                            gwf[:, w, hd : hd + 1],
                            None, MULT,
                        )
                    # elu(x) = relu(x) + exp(min(x,0)) - 1 ; h1 = mean_heads
                    neg = flp.tile([128, OW1], F32, tag="neg")
                    nc.vector.tensor_scalar(neg[:], o_t[:], 0.0, None, MIN)
                    ex = flp.tile([128, OW1], F32, tag="ex")
                    nc.scalar.activation(ex[:], neg[:], EXP)
                    rl = flp.tile([128, OW1], F32, tag="rl")
                    nc.vector.tensor_relu(rl[:], o_t[:])
                    su = flp.tile([128, OW1], F32, tag="su")
                    nc.vector.tensor_tensor(su[:], rl[:], ex[:], ADD)
                    t01 = flp.tile([128, F], F32, tag="t01")
                    nc.vector.tensor_tensor(t01[:], su[:, 0:F], su[:, F : 2 * F], ADD)
                    t23 = flp.tile([128, F], F32, tag="t23")
                    nc.vector.tensor_tensor(
                        t23[:], su[:, 2 * F : 3 * F], su[:, 3 * F :], ADD
                    )
                    h1_t = flp.tile([128, F], F32, tag="h1")
                    nc.vector.tensor_tensor(h1_t[:], t01[:], t23[:], ADD)
                    nc.vector.tensor_scalar(h1_t[:], h1_t[:], 0.25, -1.0, MULT, ADD)
                    ptt = ptw.tile([128, 128], F32, tag="ptt")
                    nc.tensor.transpose(ptt[:], h1_t[:], id2[:])
                    nc.vector.tensor_copy(h1T_sb[:, w * 128 : (w + 1) * 128], ptt[:])

                _spmm(nc, tc, bB, CHUNK, T.bidx_d, T.brelf_d, T.hh1_tab, OW1,
                      OW1, OW1, iof2, "a1" + s, flush1, skip=("agg1" in SKIP),
                      bufs=3)

            nc.sync.dma_start(T.h1T_loc[:], h1T_sb[:, 0:SLICE])
            nc.gpsimd.collective_compute(
                "AllGather", BYPASS, groups,
                ins=[T.h1T_loc[:].opt()], outs=[T.h1T_ag[:].opt()],
            )
            h1T_full = pp2.tile([128, P, SLICE], F32)
            nc.sync.dma_start(h1T_full[:], T.h1T_ag.ap().rearrange("s f n -> f s n"))
            h1T_flat = h1T_full[:].rearrange("f s n -> f (s n)")

            h2_nm = pp2.tile([128, NBLK, C], F32)
            g2_nm = pp2.tile([128, NBLK, 1], F32)
            with tc.tile_pool(name="ph2" + s, bufs=2, space="PSUM") as ph2p:
                for b in range(NBLK):
                    nv = max(0, min(128, N - b * 128))
                    if nv < 128:
                        nc.vector.memset(h2_nm[:, b, :], 0.0)
                        nc.vector.memset(g2_nm[:, b, :], 0.0)
                    if nv == 0:
                        continue
                    ph2 = ph2p.tile([128, C + 1], F32)
                    nc.tensor.matmul(
                        ph2[0:nv, :],
                        h1T_flat[:, b * 128 : b * 128 + nv],
                        W2cat[:],
                        start=True,
                        stop=True,
                    )
                    nc.vector.tensor_copy(h2_nm[0:nv, b, :], ph2[0:nv, 0:C])
                    nc.scalar.activation(g2_nm[0:nv, b, :], ph2[0:nv, C : C + 1], EXP)

            with tc.tile_pool(name="stage2" + s, bufs=1) as stp:
                st = stp.tile([128, NBLK, 128], BF16, tag="stage2")
                nc.vector.memset(st[:], 0.0)
                nc.vector.tensor_copy(st[:, :, 0:1], g2_nm[:])
                nc.sync.dma_start(
                    T.g2_tab.ap().rearrange("(b p) c -> p b c", p=128), st[:]
                )

            with tc.tile_pool(name="zu2" + s, bufs=3) as zup:

                def zflush2(w, po, zup=zup):
                    u_t = zup.tile([128, 1], F32, tag="u2")
                    nc.vector.reciprocal(u_t[:], po[:, 0:1])
                    rows = min(128, SLICE - 128 * w)
                    nc.sync.dma_start(
                        T.u2_sl[w * 128 : w * 128 + rows, :], u_t[0:rows, :]
                    )

                _spmm(nc, tc, zB, 32, T.zidx_d, T.zrel_d, T.g2_tab, 128, 8, 8,
                      iof2, "z2" + s, zflush2, skip=("z" in SKIP), bufs=3)

            nc.gpsimd.collective_compute(
                "AllGather", BYPASS, groups,
                ins=[T.u2_sl[:].opt()], outs=[T.u2_full[0:N, :].opt()],
            )
            zt2 = sp2.tile([NPAD - N, 1], F32, tag="zpad2")
            nc.vector.memset(zt2[:], 0.0)
            nc.sync.dma_start(T.u2_full[N:NPAD, :], zt2[:])

            u2_nm = pp2.tile([128, NBLK, 1], F32)
            nc.sync.dma_start(
                u2_nm[:], T.u2_full.ap().rearrange("(b p) c -> p b c", p=128)
            )
            with tc.tile_pool(name="stage3" + s, bufs=1) as stp:
                st = stp.tile([128, NBLK, 128], BF16, tag="stage3")
                nc.vector.memset(st[:], 0.0)
                for b in range(NBLK):
                    nc.vector.tensor_scalar(
                        st[:, b, 0:C], h2_nm[:, b, :], u2_nm[:, b, :], None, MULT
                    )
                nc.sync.dma_start(
                    T.hh2_tab.ap().rearrange("(b p) c -> p b c", p=128), st[:]
                )

            with (
                tc.tile_pool(name="gw2" + s, bufs=2) as gwp,
                tc.tile_pool(name="fl2" + s, bufs=2) as flp,
            ):
                gwi = gwp.tile([128, NWIN * 8], I16, tag="gwi2")
                nc.sync.dma_start(gwi[:], T.gwidx_d[:])
                gwb = gwp.tile([128, NWIN, 128], BF16, tag="gwb2")
                nc.gpsimd.dma_gather(
                    gwb[:], T.g2_tab[:], gwi[:], NWIN * 128, NWIN * 128, 128,
                    single_packet=False,
                )
                gwf = gwp.tile([128, NWIN, 128], F32, tag="gwf2")
                nc.vector.tensor_copy(gwf[:], gwb[:])

                def flush2(w, po):
                    o2 = flp.tile([128, C], F32, tag="o2")
                    nc.vector.tensor_scalar(
                        o2[:], po[:, 0:C], gwf[:, w, 0:1], None, MULT
                    )
                    rows = min(128, SLICE - 128 * w)
                    nc.sync.dma_start(
                        T.out_d[w * 128 : w * 128 + rows, :], o2[0:rows, :]
                    )

                _spmm(nc, tc, bB, 32, T.bidx_d, T.brelf_d, T.hh2_tab, 128,
                      C, C, iof2, "a2" + s, flush2, skip=("agg2" in SKIP),
                      bufs=3)



def _build_program(zB, bB, reps=1):
    nc = bacc.Bacc("TRN2", target_bir_lowering=False, debug=False, num_devices=P)
    groups = [list(range(P))]
    T = _declare(nc, zB, bB)
    with tile.TileContext(nc) as tc:
        for r in range(reps):
            _emit(nc, tc, T, zB, bB, s=str(r))
            if reps > 1:
                with tc.tile_critical():
                    nc.all_core_barrier()
    nc.compile()
    return nc


def _host_inputs(x, W1, a1, W2, a2):
    xT = np.zeros((F, NPAD), np.float32)
    xT[:, :N] = np.ascontiguousarray(np.asarray(x, np.float32).T)
    a1 = np.asarray(a1, np.float32)
    a2 = np.asarray(a2, np.float32)
    a1rc = np.ascontiguousarray(a1[:, F : 2 * F].T)  # [128, H]
    a2rc = np.zeros((F, 1), np.float32)
    a2rc[0:C, 0] = a2[0, C : 2 * C]
    iota = np.tile(np.arange(128, dtype=np.float32), (128, 1))
    return dict(
        xT=xT,
        W1=np.asarray(W1, np.float32),
        W2=np.asarray(W2, np.float32),
        a1rc=a1rc,
        a2rc=a2rc,
        ident=np.eye(128, dtype=np.float32),
        iota_f=np.ascontiguousarray(iota),
    )


def build(x, edge_index, W1, a1, W2, a2, reps=1):
    """Build program + per-core input maps. Returns (nc, in_maps)."""
    ei = np.asarray(edge_index)
    row = ei[0].astype(np.int64)
    col = ei[1].astype(np.int64)
    zB, bB, per_core = _build_edge_inputs(row, col)
    nc = _build_program(zB, bB, reps=reps)
    common = _host_inputs(x, W1, a1, W2, a2)
    in_maps = [{**common, **per_core[k]} for k in range(P)]
    return nc, in_maps


def kernel(x, edge_index, W1, a1, W2, a2):
    nc, in_maps = build(x, edge_index, W1, a1, W2, a2)
    res = run_bass_kernel_spmd(nc, in_maps, list(range(P)))
    return np.concatenate([res.results[k]["out"] for k in range(P)], axis=0)



# revision 2
# speedup vs baseline: 1.1851x; 1.1851x over previous
"""GAT (2-layer) Trainium2 kernel, SPMD across 8 NeuronCores.

Key algebra: segment softmax keyed by row is shift invariant, so the
(h[row] . a_l) term cancels and attention factorizes:
    alpha_e = g[col_e] * u[row_e],
    g[n] = exp(h[n] . a_r),   u[r] = 1 / sum_{e: row=r} g[col_e]
Each GAT layer then needs only two unweighted sparse ops over the fixed
graph:
    z   = A @ g          (segment-sum keyed by row)   -> u = 1/z
    agg = A^T @ (u * h)  (segment-sum keyed by col)
    out = g * agg
Both are done as: dma_gather of table rows per edge (128 edges/block) +
one-hot matmul (lhsT = one-hot of block-relative destination, built by a
DVE is_equal against an iota tile) accumulating into a PSUM window.

v2 layout: uniform 1280-node slices (NPAD=10240). Each core uploads ONLY
its x slice plus weights/edge metadata packed into 3 dtype-blobs (the
axon host->device link is the bottleneck: ~70 ms fixed + ~5 ms/MB +
~8.5 ms/array). Dense layers are sharded (each core computes h/g for its
slice); full gather tables are assembled on-device via AllGather of the
per-core slices. Per-edge gather indices ship 16-partition-compact and
are replicated to 128 partitions on device; rel (dst-in-window) ships as
bf16.

kernel(**inputs) takes FULL inputs and returns the FULL [10000, 22] output.
"""

import sys

sys.path.insert(0, "/opt/trn_rl_repo")

import numpy as np
import jax

# Every run_bass_kernel_spmd call re-traces and re-compiles the XLA wrapper
# (fresh closure), re-running the BIR->NEFF compile (~0.2 s/call). The JAX
# persistent compilation cache keys on the serialized HLO and skips all of
# it after the first call.
jax.config.update("jax_compilation_cache_dir", "/tmp/jax_comp_cache_gat")
jax.config.update("jax_persistent_cache_min_entry_size_bytes", -1)
jax.config.update("jax_persistent_cache_min_compile_time_secs", 0.0)

from concourse import bacc, mybir, tile
from concourse.bass_utils import run_bass_kernel_spmd

F32 = mybir.dt.float32
BF16 = mybir.dt.bfloat16
I16 = mybir.dt.int16
EXP = mybir.ActivationFunctionType.Exp
EQ = mybir.AluOpType.is_equal
MULT = mybir.AluOpType.mult
ADD = mybir.AluOpType.add
MIN = mybir.AluOpType.min
MAX = mybir.AluOpType.max
BYPASS = mybir.AluOpType.bypass

N = 10000
E = 320000
F = 128
H = 4
C = 22
P = 8
SLICE = 1280                 # nodes per core (core 7: 1040 real + 240 pad)
NPAD = P * SLICE             # 10240
NWIN = SLICE // 128          # 10
OW1 = H * F                  # 512
DUMMY = NPAD - 1             # pad node; all table rows there are zero
EPS = 1e-20
CH_Z = 32                    # gather chunk (blocks) for z phases
CH_A1 = 16                   # gather chunk for layer-1 aggregation
CH_A2 = 32

# bfblob column offsets (all bf16)
_XO = 0
_WO = _XO + SLICE            # W1
_W2O = _WO + OW1
_A1O = _W2O + C
_A2O = _A1O + H
_MO = _A2O + 1               # mask [128, NWIN]
_IFO = _MO + NWIN            # iota_f [128, 128]
_IPO = _IFO + 128            # iota_p [128, 1]
_RZO = _IPO + 1              # zrel [128, ZT_z]


def _cdiv(a, b):
    return (a + b - 1) // b


def _wrap_idxs(idx):
    """dma_gather index layout: logical i at [i%16, i//16], 16 partitions
    (replicated to 128 on device)."""
    n = idx.shape[0]
    assert n % 16 == 0
    return np.ascontiguousarray(idx.reshape(n // 16, 16).T.astype(np.int16))


def _phase_arrays(key, other, nwin):
    """Group one core's (already core-local) edges by 128-wide key window.
    Returns per-window (rel, other) with rel = key - 128*w."""
    w = key >> 7
    order = np.argsort(w, kind="stable")
    key, other, w = key[order], other[order], w[order]
    out = []
    bounds = np.searchsorted(w, np.arange(nwin + 1))
    for i in range(nwin):
        sl = slice(bounds[i], bounds[i + 1])
        k, o = key[sl] - 128 * i, other[sl]
        so = np.argsort(o, kind="stable")  # sorted gather idx -> HBM locality
        out.append((k[so], o[so]))
    return out


def _build_edge_inputs(row, col):
    zraw, braw = [], []
    for k in range(P):
        base = k * SLICE
        m = (row >= base) & (row < base + SLICE)
        zraw.append(_phase_arrays(row[m] - base, col[m], NWIN))
        m = (col >= base) & (col < base + SLICE)
        braw.append(_phase_arrays(col[m] - base, row[m], NWIN))

    def block_counts(raw):
        return [
            max(_cdiv(max(max(len(raw[k][w][0]) for k in range(P)), 1), 128), 1)
            for w in range(NWIN)
        ]

    zB = block_counts(zraw)
    bB = block_counts(braw)

    def pack(raw, B):
        idx_l, rel_l = [], []
        for w in range(NWIN):
            n = B[w] * 128
            rel = np.zeros(n, np.int32)
            oth = np.full(n, DUMMY, np.int32)  # dummy -> zero table row
            r, o = raw[w]
            rel[: len(r)] = r
            oth[: len(o)] = o
            idx_l.append(_wrap_idxs(oth))
            rel_l.append(
                rel.reshape(B[w], 128).T.astype(np.float32)
            )
        import ml_dtypes

        return (
            np.concatenate(idx_l, 1),
            np.concatenate(rel_l, 1).astype(ml_dtypes.bfloat16),
        )

    per_core = []
    for k in range(P):
        zidx, zrel = pack(zraw[k], zB)
        bidx, brel = pack(braw[k], bB)
        per_core.append((zidx, zrel, bidx, brel))
    return zB, bB, per_core


def _spmm(nc, tc, B, CH, idx_t, idx_off, rel_t, rel_off, tab, elem, rhs_w,
          psum_w, iof_t, name, flush, bufs=3):
    """One-hot-matmul SpMM over 128-dst windows with gather chunks that span
    window boundaries. idx_t/rel_t are persistent SBUF tiles holding the
    whole phase's indices (replicated) / rel values (f32).
    flush(w, po) consumes each window's PSUM result."""
    with (
        tc.tile_pool(name=f"gg{name}", bufs=bufs) as ggp,
        tc.tile_pool(name=f"go{name}", bufs=bufs) as ohp,
        tc.tile_pool(name=f"gp{name}", bufs=2, space="PSUM") as pp,
    ):
        total = sum(B)
        gts, ohs = {}, {}
        gb = 0
        for w, Bw in enumerate(B):
            po = pp.tile([128, psum_w], F32, tag="po")
            for b in range(Bw):
                ch, off = divmod(gb, CH)
                if off == 0:
                    cb = min(CH, total - ch * CH)
                    gt = ggp.tile([128, CH, elem], BF16, tag="gg")
                    nc.gpsimd.dma_gather(
                        gt[:, :cb, :], tab[:],
                        idx_t[:, idx_off + ch * CH * 8 : idx_off + (ch * CH + cb) * 8],
                        cb * 128, cb * 128, elem, single_packet=False,
                    )
                    oh = ohp.tile([128, CH, 128], BF16, tag="go")
                    nc.vector.tensor_tensor(
                        oh[:, :cb, :],
                        iof_t[:].rearrange("p (x f) -> p x f", x=1)
                        .broadcast_to([128, cb, 128]),
                        rel_t[:, rel_off + ch * CH : rel_off + ch * CH + cb]
                        .rearrange("p (b x) -> p b x", x=1)
                        .broadcast_to([128, cb, 128]),
                        EQ,
                    )
                    gts[ch], ohs[ch] = gt, oh
                nc.tensor.matmul(
                    po[:], ohs[ch][:, off, :], gts[ch][:, off, 0:rhs_w],
                    start=(b == 0), stop=(b == Bw - 1),
                )
                gb += 1
            flush(w, po)


def _declare(nc, ZT_z, ZT_b):
    T = type("T", (), {})()
    T.bfblob = nc.dram_tensor(
        "bfblob", [128, _RZO + ZT_z + ZT_b], BF16, kind="ExternalInput"
    )
    T.iblob = nc.dram_tensor(
        "iblob", [16, (ZT_z + ZT_b) * 8], I16, kind="ExternalInput"
    )
    T.out_d = nc.dram_tensor("out", [SLICE, C], F32, kind="ExternalOutput")

    T.g1_sl = nc.dram_tensor("g1_sl", [SLICE, 128], BF16)
    T.g1_tab = nc.dram_tensor("g1_tab", [NPAD, 128], BF16, addr_space="Shared")
    T.hh1_sl = nc.dram_tensor("hh1_sl", [SLICE, OW1], BF16)
    T.hh1_tab = nc.dram_tensor("hh1_tab", [NPAD, OW1], BF16, addr_space="Shared")
    T.g2_sl = nc.dram_tensor("g2_sl", [SLICE, 128], BF16)
    T.g2_tab = nc.dram_tensor("g2_tab", [NPAD, 128], BF16, addr_space="Shared")
    T.hh2_sl = nc.dram_tensor("hh2_sl", [SLICE, 128], BF16)
    T.hh2_tab = nc.dram_tensor("hh2_tab", [NPAD, 128], BF16, addr_space="Shared")
    return T


def _emit(nc, tc, T, zB, bB, s=""):
    groups = [list(range(P))]
    ZT_z, ZT_b = sum(zB), sum(bB)
    with (
        tc.tile_pool(name="persist" + s, bufs=1) as pp,
        tc.tile_pool(name="small" + s, bufs=3) as sp,
    ):
        # ---------------- parameter / metadata load ----------------
        W1_t = pp.tile([F, OW1], BF16)
        nc.sync.dma_start(W1_t[:], T.bfblob[:, _WO : _WO + OW1])
        w2bf = sp.tile([F, C], BF16, tag="w2bf")
        nc.sync.dma_start(w2bf[:], T.bfblob[:, _W2O : _W2O + C])
        W2cat = pp.tile([F, C + 1], F32)
        nc.vector.tensor_copy(W2cat[:, 0:C], w2bf[:])
        a1bf = sp.tile([F, H], BF16, tag="a1bf")
        nc.sync.dma_start(a1bf[:], T.bfblob[:, _A1O : _A1O + H])
        a1rc_t = pp.tile([F, H], F32)
        nc.vector.tensor_copy(a1rc_t[:], a1bf[:])
        a2bf = sp.tile([F, 1], BF16, tag="a2bf")
        nc.sync.dma_start(a2bf[:], T.bfblob[:, _A2O : _A2O + 1])
        a2rc_t = pp.tile([F, 1], F32)
        nc.vector.tensor_copy(a2rc_t[:], a2bf[:])
        mbf = sp.tile([F, NWIN], BF16, tag="mbf")
        nc.sync.dma_start(mbf[:], T.bfblob[:, _MO : _MO + NWIN])
        mask_t = pp.tile([F, NWIN], F32)
        nc.vector.tensor_copy(mask_t[:], mbf[:])
        iofb = sp.tile([128, 128], BF16, tag="iofb")
        nc.sync.dma_start(iofb[:], T.bfblob[:, _IFO : _IFO + 128])
        iof_t = pp.tile([128, 128], F32)
        nc.vector.tensor_copy(iof_t[:], iofb[:])
        iopb = sp.tile([128, 1], BF16, tag="iopb")
        nc.sync.dma_start(iopb[:], T.bfblob[:, _IPO : _IPO + 1])
        iop_t = sp.tile([128, 1], F32, tag="iop")
        nc.vector.tensor_copy(iop_t[:], iopb[:])
        id_t = pp.tile([128, 128], F32)
        nc.vector.tensor_scalar(id_t[:], iof_t[:], iop_t[:, 0:1], None, EQ)
        W1f = pp.tile([F, OW1], F32)
        nc.vector.tensor_copy(W1f[:], W1_t[:])

        zidx_t = pp.tile([128, ZT_z * 8], I16)
        bidx_t = pp.tile([128, ZT_b * 8], I16)
        for g in range(8):
            nc.sync.dma_start(
                zidx_t[16 * g : 16 * g + 16, :], T.iblob[:, 0 : ZT_z * 8]
            )
            nc.sync.dma_start(
                bidx_t[16 * g : 16 * g + 16, :],
                T.iblob[:, ZT_z * 8 : (ZT_z + ZT_b) * 8],
            )
        rel_bf = sp.tile([128, ZT_z + ZT_b], BF16, tag="relbf")
        nc.sync.dma_start(rel_bf[:], T.bfblob[:, _RZO : _RZO + ZT_z + ZT_b])
        rel_t = pp.tile([128, ZT_z + ZT_b], F32)
        nc.vector.tensor_copy(rel_t[:], rel_bf[:])

        # ---------------- W1ar / W2cat attn columns ----------------
        W1ar_t = pp.tile([F, H], F32)
        W1arb = pp.tile([F, H], BF16)
        with tc.tile_pool(name="ptr" + s, bufs=2, space="PSUM") as ptr:
            for hd in range(H):
                pt = ptr.tile([128, 128], F32, tag="pt")
                nc.tensor.transpose(pt[:], W1f[:, hd * F : (hd + 1) * F], id_t[:])
                w1t = sp.tile([128, 128], F32, tag="w1t")
                nc.vector.tensor_copy(w1t[:], pt[:])
                pv = ptr.tile([128, 1], F32, tag="pv")
                nc.tensor.matmul(
                    pv[:], w1t[:], a1rc_t[:, hd : hd + 1], start=True, stop=True
                )
                nc.vector.tensor_copy(W1ar_t[:, hd : hd + 1], pv[:])
            nc.vector.tensor_copy(W1arb[:], W1ar_t[:])
            pt2 = ptr.tile([128, 128], F32, tag="pt")
            nc.tensor.transpose(pt2[0:C, :], W2cat[:, 0:C], id_t[:])
            w2t = sp.tile([128, 128], F32, tag="w1t")
            nc.vector.tensor_copy(w2t[0:C, :], pt2[0:C, :])
            pv2 = ptr.tile([128, 1], F32, tag="pv")
            nc.tensor.matmul(
                pv2[:], w2t[0:C, :], a2rc_t[0:C, :], start=True, stop=True
            )
            nc.vector.tensor_copy(W2cat[:, C : C + 1], pv2[:])

        # ---------------- dense layer 1 (local slice only) ----------------
        h_nm = pp.tile([128, NWIN, OW1], F32)
        g1_nm = pp.tile([128, NWIN, H], F32)
        with (
            tc.tile_pool(name="xtp" + s, bufs=3) as xtp,
            tc.tile_pool(name="ph" + s, bufs=2, space="PSUM") as php,
            tc.tile_pool(name="psr" + s, bufs=2, space="PSUM") as psrp,
        ):
            for b in range(NWIN):
                xt = xtp.tile([128, 128], BF16)
                nc.sync.dma_start(
                    xt[:], T.bfblob[:, _XO + b * 128 : _XO + (b + 1) * 128]
                )
                ph = php.tile([128, OW1], F32)
                nc.tensor.matmul(ph[:], xt[:], W1_t[:], start=True, stop=True)
                psr = psrp.tile([128, H], F32)
                nc.tensor.matmul(psr[:], xt[:], W1arb[:], start=True, stop=True)
                nc.vector.tensor_copy(h_nm[:, b, :], ph[:])
                nc.scalar.activation(g1_nm[:, b, :], psr[:], EXP)

        # ---------------- g1 table slice + AllGather ----------------
        with tc.tile_pool(name="stage1" + s, bufs=1) as stp:
            st = stp.tile([128, NWIN, 128], BF16, tag="stg1")
            nc.vector.memset(st[:], 0.0)
            for b in range(NWIN):
                nc.vector.tensor_scalar(
                    st[:, b, 0:H], g1_nm[:, b, :], mask_t[:, b : b + 1], None, MULT
                )
            nc.sync.dma_start(
                T.g1_sl.ap().rearrange("(b p) c -> p b c", p=128), st[:]
            )
        nc.gpsimd.collective_compute(
            "AllGather", BYPASS, groups,
            ins=[T.g1_sl[:].opt()], outs=[T.g1_tab[:].opt()],
        )

        # ---------------- z1 ----------------
        u1_nm = pp.tile([128, NWIN, H], F32)

        def zflush1(w, po):
            zc = sp.tile([128, H], F32, tag="zc")
            nc.vector.tensor_scalar(zc[:], po[:, 0:H], EPS, None, MAX)
            nc.vector.reciprocal(u1_nm[:, w, :], zc[:])

        _spmm(nc, tc, zB, CH_Z, zidx_t, 0, rel_t, 0, T.g1_tab, 128, 8, 8,
              iof_t, "z1" + s, zflush1, bufs=2)

        # ---------------- hh1 table slice + AllGather ----------------
        with tc.tile_pool(name="hhp" + s, bufs=3) as hhp:
            for b in range(NWIN):
                hh = hhp.tile([128, OW1], BF16)
                for hd in range(H):
                    nc.vector.tensor_scalar(
                        hh[:, hd * F : (hd + 1) * F],
                        h_nm[:, b, hd * F : (hd + 1) * F],
                        u1_nm[:, b, hd : hd + 1],
                        None,
                        MULT,
                    )
                nc.sync.dma_start(
                    T.hh1_sl.ap().rearrange("(b p) c -> p b c", p=128)[:, b, :],
                    hh[:],
                )
        nc.gpsimd.collective_compute(
            "AllGather", BYPASS, groups,
            ins=[T.hh1_sl[:].opt()], outs=[T.hh1_tab[:].opt()],
        )

        # ---------------- agg1 (+ ELU + head mean + transpose) ----------------
        h1T_sb = pp.tile([128, SLICE], F32)
        with (
            tc.tile_pool(name="ptw" + s, bufs=2, space="PSUM") as ptw,
            tc.tile_pool(name="flush" + s, bufs=2) as flp,
        ):
            def flush1(w, po):
                o_t = flp.tile([128, OW1], F32, tag="o")
                for hd in range(H):
                    nc.vector.tensor_scalar(
                        o_t[:, hd * F : (hd + 1) * F],
                        po[:, hd * F : (hd + 1) * F],
                        g1_nm[:, w, hd : hd + 1],
                        None, MULT,
                    )
                # elu(x) = relu(x) + exp(min(x,0)) - 1 ; h1 = mean_heads
                neg = flp.tile([128, OW1], F32, tag="neg")
                nc.vector.tensor_scalar(neg[:], o_t[:], 0.0, None, MIN)
                ex = flp.tile([128, OW1], F32, tag="ex")
                nc.scalar.activation(ex[:], neg[:], EXP)
                rl = flp.tile([128, OW1], F32, tag="rl")
                nc.vector.tensor_relu(rl[:], o_t[:])
                su = flp.tile([128, OW1], F32, tag="su")
                nc.vector.tensor_tensor(su[:], rl[:], ex[:], ADD)
                t01 = flp.tile([128, F], F32, tag="t01")
                nc.vector.tensor_tensor(t01[:], su[:, 0:F], su[:, F : 2 * F], ADD)
                t23 = flp.tile([128, F], F32, tag="t23")
                nc.vector.tensor_tensor(
                    t23[:], su[:, 2 * F : 3 * F], su[:, 3 * F :], ADD
                )
                h1_t = flp.tile([128, F], F32, tag="h1")
                nc.vector.tensor_tensor(h1_t[:], t01[:], t23[:], ADD)
                nc.vector.tensor_scalar(h1_t[:], h1_t[:], 0.25, -1.0, MULT, ADD)
                ptt = ptw.tile([128, 128], F32, tag="ptt")
                nc.tensor.transpose(ptt[:], h1_t[:], id_t[:])
                nc.vector.tensor_copy(h1T_sb[:, w * 128 : (w + 1) * 128], ptt[:])

            _spmm(nc, tc, bB, CH_A1, bidx_t, 0, rel_t, ZT_z, T.hh1_tab, OW1,
                  OW1, OW1, iof_t, "a1" + s, flush1, bufs=3)

        # ---------------- dense layer 2 (local slice) ----------------
        h2_nm = pp.tile([128, NWIN, C], F32)
        g2_nm = pp.tile([128, NWIN, 1], F32)
        with tc.tile_pool(name="ph2" + s, bufs=2, space="PSUM") as ph2p:
            for b in range(NWIN):
                ph2 = ph2p.tile([128, C + 1], F32)
                nc.tensor.matmul(
                    ph2[:], h1T_sb[:, b * 128 : (b + 1) * 128], W2cat[:],
                    start=True, stop=True,
                )
                nc.vector.tensor_copy(h2_nm[:, b, :], ph2[:, 0:C])
                nc.scalar.activation(g2_nm[:, b, :], ph2[:, C : C + 1], EXP)

        # ---------------- g2 table slice + AllGather ----------------
        with tc.tile_pool(name="stage2" + s, bufs=1) as stp:
            st = stp.tile([128, NWIN, 128], BF16, tag="stg2")
            nc.vector.memset(st[:], 0.0)
            for b in range(NWIN):
                nc.vector.tensor_scalar(
                    st[:, b, 0:1], g2_nm[:, b, :], mask_t[:, b : b + 1], None, MULT
                )
            nc.sync.dma_start(
                T.g2_sl.ap().rearrange("(b p) c -> p b c", p=128), st[:]
            )
        nc.gpsimd.collective_compute(
            "AllGather", BYPASS, groups,
            ins=[T.g2_sl[:].opt()], outs=[T.g2_tab[:].opt()],
        )

        # ---------------- z2 ----------------
        u2_nm = pp.tile([128, NWIN, 1], F32)

        def zflush2(w, po):
            zc = sp.tile([128, 1], F32, tag="zc2")
            nc.vector.tensor_scalar(zc[:], po[:, 0:1], EPS, None, MAX)
            nc.vector.reciprocal(u2_nm[:, w, :], zc[:])

        _spmm(nc, tc, zB, CH_Z, zidx_t, 0, rel_t, 0, T.g2_tab, 128, 8, 8,
              iof_t, "z2" + s, zflush2, bufs=3)

        # ---------------- hh2 table slice + AllGather ----------------
        with tc.tile_pool(name="stage3" + s, bufs=1) as stp:
            st = stp.tile([128, NWIN, 128], BF16, tag="stg3")
            nc.vector.memset(st[:], 0.0)
            for b in range(NWIN):
                nc.vector.tensor_scalar(
                    st[:, b, 0:C], h2_nm[:, b, :], u2_nm[:, b, 0:1], None, MULT
                )
            nc.sync.dma_start(
                T.hh2_sl.ap().rearrange("(b p) c -> p b c", p=128), st[:]
            )
        nc.gpsimd.collective_compute(
            "AllGather", BYPASS, groups,
            ins=[T.hh2_sl[:].opt()], outs=[T.hh2_tab[:].opt()],
        )

        # ---------------- agg2 -> output ----------------
        with tc.tile_pool(name="fl2" + s, bufs=2) as flp:

            def flush2(w, po):
                o2 = flp.tile([128, C], F32, tag="o2")
                nc.vector.tensor_scalar(
                    o2[:], po[:, 0:C], g2_nm[:, w, 0:1], None, MULT
                )
                nc.sync.dma_start(
                    T.out_d[w * 128 : (w + 1) * 128, :], o2[:]
                )

            _spmm(nc, tc, bB, CH_A2, bidx_t, 0, rel_t, ZT_z, T.hh2_tab, 128,
                  C, C, iof_t, "a2" + s, flush2, bufs=3)


def _build_program(zB, bB, reps=1):
    nc = bacc.Bacc("TRN2", target_bir_lowering=False, debug=False, num_devices=P)
    T = _declare(nc, sum(zB), sum(bB))
    with tile.TileContext(nc) as tc:
        for r in range(reps):
            _emit(nc, tc, T, zB, bB, s=str(r) if reps > 1 else "")
            if reps > 1:
                with tc.tile_critical():
                    nc.all_core_barrier()
    nc.compile()
    return nc


def _host_inputs(x, W1, a1, W2, a2, per_core):
    import ml_dtypes

    BF = ml_dtypes.bfloat16
    xT = np.zeros((F, NPAD), np.float32)
    xT[:, :N] = np.ascontiguousarray(np.asarray(x, np.float32).T)
    a1 = np.asarray(a1, np.float32)
    a2 = np.asarray(a2, np.float32)
    a1rc = np.ascontiguousarray(a1[:, F : 2 * F].T)  # [128, H]
    a2rc = np.zeros((F, 1), np.float32)
    a2rc[0:C, 0] = a2[0, C : 2 * C]
    W1 = np.asarray(W1, np.float32)
    W2 = np.asarray(W2, np.float32)
    iota_f = np.tile(np.arange(128, dtype=np.float32), (128, 1))
    iota_p = np.arange(128, dtype=np.float32).reshape(128, 1)
    ids = np.arange(NPAD)
    in_maps = []
    for k in range(P):
        base = k * SLICE
        mask = (
            (ids[base : base + SLICE] < N)
            .astype(np.float32)
            .reshape(NWIN, 128)
            .T
        )
        zidx, zrel, bidx, brel = per_core[k]
        bfb = np.concatenate(
            [
                xT[:, base : base + SLICE].astype(BF),
                W1.astype(BF), W2.astype(BF), a1rc.astype(BF),
                a2rc.astype(BF), np.ascontiguousarray(mask).astype(BF),
                iota_f.astype(BF), iota_p.astype(BF),
                np.asarray(zrel, BF), np.asarray(brel, BF),
            ],
            axis=1,
        )
        ib = np.ascontiguousarray(np.concatenate([zidx, bidx], axis=1))
        in_maps.append(dict(bfblob=bfb, iblob=ib))
    return in_maps


def build(x, edge_index, W1, a1, W2, a2, reps=1):
    """Build program + per-core input maps. Returns (nc, in_maps)."""
    ei = np.asarray(edge_index)
    row = ei[0].astype(np.int64)
    col = ei[1].astype(np.int64)
    zB, bB, per_core = _build_edge_inputs(row, col)
    nc = _build_program(zB, bB, reps=reps)
    in_maps = _host_inputs(x, W1, a1, W2, a2, per_core)
    return nc, in_maps


def kernel(x, edge_index, W1, a1, W2, a2):
    nc, in_maps = build(x, edge_index, W1, a1, W2, a2)
    res = run_bass_kernel_spmd(nc, in_maps, list(range(P)))
    out = np.concatenate([res.results[k]["out"] for k in range(P)], axis=0)
    return out[:N]


# revision 3
# speedup vs baseline: 1.2614x; 1.0644x over previous
"""GAT (2-layer) Trainium2 kernel, SPMD across 8 NeuronCores.

Key algebra: segment softmax keyed by row is shift invariant, so the
(h[row] . a_l) term cancels and attention factorizes:
    alpha_e = g[col_e] * u[row_e],
    g[n] = exp(h[n] . a_r),   u[r] = 1 / sum_{e: row=r} g[col_e]
Each GAT layer then needs only two unweighted sparse ops over the fixed
graph:
    z   = A @ g          (segment-sum keyed by row)   -> u = 1/z
    agg = A^T @ (u * h)  (segment-sum keyed by col)
    out = g * agg
Both are done as: dma_gather of table rows per edge (128 edges/block) +
one-hot matmul (lhsT = one-hot of block-relative destination, built by a
DVE is_equal against an iota tile) accumulating into a PSUM window.

v2 layout: uniform 1280-node slices (NPAD=10240). Each core uploads ONLY
its x slice plus weights/edge metadata packed into 3 dtype-blobs (the
axon host->device link is the bottleneck: ~70 ms fixed + ~5 ms/MB +
~8.5 ms/array). Dense layers are sharded (each core computes h/g for its
slice); full gather tables are assembled on-device via AllGather of the
per-core slices. Per-edge gather indices ship 16-partition-compact and
are replicated to 128 partitions on device; rel (dst-in-window) ships as
bf16.

kernel(**inputs) takes FULL inputs and returns the FULL [10000, 22] output.
"""

import sys

sys.path.insert(0, "/opt/trn_rl_repo")

import numpy as np
import jax

# Every run_bass_kernel_spmd call re-traces and re-compiles the XLA wrapper
# (fresh closure), re-running the BIR->NEFF compile (~0.2 s/call). The JAX
# persistent compilation cache keys on the serialized HLO and skips all of
# it after the first call.
jax.config.update("jax_compilation_cache_dir", "/tmp/jax_comp_cache_gat")
jax.config.update("jax_persistent_cache_min_entry_size_bytes", -1)
jax.config.update("jax_persistent_cache_min_compile_time_secs", 0.0)

from concourse import bacc, mybir, tile
from concourse.bass_utils import run_bass_kernel_spmd

F32 = mybir.dt.float32
BF16 = mybir.dt.bfloat16
I16 = mybir.dt.int16
EXP = mybir.ActivationFunctionType.Exp
EQ = mybir.AluOpType.is_equal
MULT = mybir.AluOpType.mult
ADD = mybir.AluOpType.add
MIN = mybir.AluOpType.min
MAX = mybir.AluOpType.max
BYPASS = mybir.AluOpType.bypass

N = 10000
E = 320000
F = 128
H = 4
C = 22
P = 8
SLICE = 1280                 # nodes per core (core 7: 1040 real + 240 pad)
NPAD = P * SLICE             # 10240
NWIN = SLICE // 128          # 10
OW1 = H * F                  # 512
DUMMY = NPAD - 1             # pad node; all table rows there are zero
EPS = 1e-20
CH_Z = 32                    # gather chunk (blocks) for z phases
CH_A1 = 16                   # gather chunk for layer-1 aggregation
CH_A2 = 32

# bfblob column offsets (all bf16)
_XO = 0
_WO = _XO + SLICE            # W1
_W2O = _WO + OW1
_A1O = _W2O + C
_A2O = _A1O + H
_MO = _A2O + 1               # mask [128, NWIN]
_IFO = _MO + NWIN            # iota_f [128, 128]
_IPO = _IFO + 128            # iota_p [128, 1]
_RZO = _IPO + 1              # zrel [128, ZT_z]


def _cdiv(a, b):
    return (a + b - 1) // b


def _wrap_idxs(idx):
    """dma_gather index layout: logical i at [i%16, i//16], 16 partitions
    (replicated to 128 on device)."""
    n = idx.shape[0]
    assert n % 16 == 0
    return np.ascontiguousarray(idx.reshape(n // 16, 16).T.astype(np.int16))


def _phase_arrays(key, other, nwin):
    """Group one core's (already core-local) edges by 128-wide key window.
    Returns per-window (rel, other) with rel = key - 128*w."""
    w = key >> 7
    order = np.argsort(w, kind="stable")
    key, other, w = key[order], other[order], w[order]
    out = []
    bounds = np.searchsorted(w, np.arange(nwin + 1))
    for i in range(nwin):
        sl = slice(bounds[i], bounds[i + 1])
        k, o = key[sl] - 128 * i, other[sl]
        so = np.argsort(o, kind="stable")  # sorted gather idx -> HBM locality
        out.append((k[so], o[so]))
    return out


def _build_edge_inputs(row, col):
    zraw, braw = [], []
    for k in range(P):
        base = k * SLICE
        m = (row >= base) & (row < base + SLICE)
        zraw.append(_phase_arrays(row[m] - base, col[m], NWIN))
        m = (col >= base) & (col < base + SLICE)
        braw.append(_phase_arrays(col[m] - base, row[m], NWIN))

    def block_counts(raw):
        return [
            max(_cdiv(max(max(len(raw[k][w][0]) for k in range(P)), 1), 128), 1)
            for w in range(NWIN)
        ]

    zB = block_counts(zraw)
    bB = block_counts(braw)

    def pack(raw, B):
        idx_l, rel_l = [], []
        for w in range(NWIN):
            n = B[w] * 128
            rel = np.zeros(n, np.int32)
            oth = np.full(n, DUMMY, np.int32)  # dummy -> zero table row
            r, o = raw[w]
            rel[: len(r)] = r
            oth[: len(o)] = o
            idx_l.append(_wrap_idxs(oth))
            rel_l.append(
                rel.reshape(B[w], 128).T.astype(np.float32)
            )
        import ml_dtypes

        return (
            np.concatenate(idx_l, 1),
            np.concatenate(rel_l, 1).astype(ml_dtypes.bfloat16),
        )

    per_core = []
    for k in range(P):
        zidx, zrel = pack(zraw[k], zB)
        bidx, brel = pack(braw[k], bB)
        per_core.append((zidx, zrel, bidx, brel))
    return zB, bB, per_core


def _spmm(nc, tc, B, CH, idx_t, idx_off, rel_t, rel_off, tab, elem, rhs_w,
          psum_w, iof_t, name, flush, bufs=3):
    """One-hot-matmul SpMM over 128-dst windows with gather chunks that span
    window boundaries. idx_t/rel_t are persistent SBUF tiles holding the
    whole phase's indices (replicated) / rel values (f32).
    flush(w, po) consumes each window's PSUM result."""
    with (
        tc.tile_pool(name=f"gg{name}", bufs=bufs) as ggp,
        tc.tile_pool(name=f"go{name}", bufs=bufs) as ohp,
        tc.tile_pool(name=f"gp{name}", bufs=2, space="PSUM") as pp,
    ):
        total = sum(B)
        gts, ohs = {}, {}
        gb = 0
        for w, Bw in enumerate(B):
            po = pp.tile([128, psum_w], F32, tag="po")
            for b in range(Bw):
                ch, off = divmod(gb, CH)
                if off == 0:
                    cb = min(CH, total - ch * CH)
                    gt = ggp.tile([128, CH, elem], BF16, tag="gg")
                    nc.gpsimd.dma_gather(
                        gt[:, :cb, :], tab[:],
                        idx_t[:, idx_off + ch * CH * 8 : idx_off + (ch * CH + cb) * 8],
                        cb * 128, cb * 128, elem, single_packet=False,
                    )
                    oh = ohp.tile([128, CH, 128], BF16, tag="go")
                    nc.vector.tensor_tensor(
                        oh[:, :cb, :],
                        iof_t[:].rearrange("p (x f) -> p x f", x=1)
                        .broadcast_to([128, cb, 128]),
                        rel_t[:, rel_off + ch * CH : rel_off + ch * CH + cb]
                        .rearrange("p (b x) -> p b x", x=1)
                        .broadcast_to([128, cb, 128]),
                        EQ,
                    )
                    gts[ch], ohs[ch] = gt, oh
                nc.tensor.matmul(
                    po[:], ohs[ch][:, off, :], gts[ch][:, off, 0:rhs_w],
                    start=(b == 0), stop=(b == Bw - 1),
                )
                gb += 1
            flush(w, po)


def _declare(nc, ZT_z, ZT_b):
    # bfblob trailing sections: rel (ZT_z+ZT_b cols) then wrapped gather
    # indices (int16 bits stored as bf16; ZT_z+ZT_b cols, rows q*16+j hold
    # wrapped[j, q*W+w] so one strided AP reconstructs the [16, ZT*8] form).
    T = type("T", (), {})()
    T.bfblob = nc.dram_tensor(
        "bfblob", [128, _RZO + 2 * (ZT_z + ZT_b)], BF16, kind="ExternalInput"
    )
    T.out_d = nc.dram_tensor("out", [SLICE, C], BF16, kind="ExternalOutput")

    T.g1_sl = nc.dram_tensor("g1_sl", [SLICE, 128], BF16)
    T.g1_tab = nc.dram_tensor("g1_tab", [NPAD, 128], BF16, addr_space="Shared")
    T.hh1_sl = nc.dram_tensor("hh1_sl", [SLICE, OW1], BF16)
    T.hh1_tab = nc.dram_tensor("hh1_tab", [NPAD, OW1], BF16, addr_space="Shared")
    T.g2_sl = nc.dram_tensor("g2_sl", [SLICE, 128], BF16)
    T.g2_tab = nc.dram_tensor("g2_tab", [NPAD, 128], BF16, addr_space="Shared")
    T.hh2_sl = nc.dram_tensor("hh2_sl", [SLICE, 128], BF16)
    T.hh2_tab = nc.dram_tensor("hh2_tab", [NPAD, 128], BF16, addr_space="Shared")
    return T


def _emit(nc, tc, T, zB, bB, s=""):
    groups = [list(range(P))]
    ZT_z, ZT_b = sum(zB), sum(bB)
    with (
        tc.tile_pool(name="persist" + s, bufs=1) as pp,
        tc.tile_pool(name="small" + s, bufs=3) as sp,
    ):
        # ---------------- parameter / metadata load ----------------
        W1_t = pp.tile([F, OW1], BF16)
        nc.sync.dma_start(W1_t[:], T.bfblob[:, _WO : _WO + OW1])
        w2bf = sp.tile([F, C], BF16, tag="w2bf")
        nc.sync.dma_start(w2bf[:], T.bfblob[:, _W2O : _W2O + C])
        W2cat = pp.tile([F, C + 1], F32)
        nc.vector.tensor_copy(W2cat[:, 0:C], w2bf[:])
        a1bf = sp.tile([F, H], BF16, tag="a1bf")
        nc.sync.dma_start(a1bf[:], T.bfblob[:, _A1O : _A1O + H])
        a1rc_t = pp.tile([F, H], F32)
        nc.vector.tensor_copy(a1rc_t[:], a1bf[:])
        a2bf = sp.tile([F, 1], BF16, tag="a2bf")
        nc.sync.dma_start(a2bf[:], T.bfblob[:, _A2O : _A2O + 1])
        a2rc_t = pp.tile([F, 1], F32)
        nc.vector.tensor_copy(a2rc_t[:], a2bf[:])
        mbf = sp.tile([F, NWIN], BF16, tag="mbf")
        nc.sync.dma_start(mbf[:], T.bfblob[:, _MO : _MO + NWIN])
        mask_t = pp.tile([F, NWIN], F32)
        nc.vector.tensor_copy(mask_t[:], mbf[:])
        iofb = sp.tile([128, 128], BF16, tag="iofb")
        nc.sync.dma_start(iofb[:], T.bfblob[:, _IFO : _IFO + 128])
        iof_t = pp.tile([128, 128], F32)
        nc.vector.tensor_copy(iof_t[:], iofb[:])
        iopb = sp.tile([128, 1], BF16, tag="iopb")
        nc.sync.dma_start(iopb[:], T.bfblob[:, _IPO : _IPO + 1])
        iop_t = sp.tile([128, 1], F32, tag="iop")
        nc.vector.tensor_copy(iop_t[:], iopb[:])
        id_t = pp.tile([128, 128], F32)
        nc.vector.tensor_scalar(id_t[:], iof_t[:], iop_t[:, 0:1], None, EQ)
        W1f = pp.tile([F, OW1], F32)
        nc.vector.tensor_copy(W1f[:], W1_t[:])

        zidx_t = pp.tile([128, ZT_z * 8], I16)
        bidx_t = pp.tile([128, ZT_b * 8], I16)
        IXZ = _RZO + ZT_z + ZT_b
        IXB = IXZ + ZT_z
        src_z = (
            T.bfblob[:, IXZ : IXZ + ZT_z]
            .rearrange("(q j) w -> j q w", q=8)
            .bitcast(I16)
        )
        src_b = (
            T.bfblob[:, IXB : IXB + ZT_b]
            .rearrange("(q j) w -> j q w", q=8)
            .bitcast(I16)
        )
        for g in range(8):
            nc.sync.dma_start(
                zidx_t[16 * g : 16 * g + 16, :].rearrange(
                    "p (q w) -> p q w", q=8
                ),
                src_z,
            )
            nc.sync.dma_start(
                bidx_t[16 * g : 16 * g + 16, :].rearrange(
                    "p (q w) -> p q w", q=8
                ),
                src_b,
            )
        rel_bf = sp.tile([128, ZT_z + ZT_b], BF16, tag="relbf")
        nc.sync.dma_start(rel_bf[:], T.bfblob[:, _RZO : _RZO + ZT_z + ZT_b])
        rel_t = pp.tile([128, ZT_z + ZT_b], F32)
        nc.vector.tensor_copy(rel_t[:], rel_bf[:])

        # ---------------- W1ar / W2cat attn columns ----------------
        W1ar_t = pp.tile([F, H], F32)
        W1arb = pp.tile([F, H], BF16)
        with tc.tile_pool(name="ptr" + s, bufs=2, space="PSUM") as ptr:
            for hd in range(H):
                pt = ptr.tile([128, 128], F32, tag="pt")
                nc.tensor.transpose(pt[:], W1f[:, hd * F : (hd + 1) * F], id_t[:])
                w1t = sp.tile([128, 128], F32, tag="w1t")
                nc.vector.tensor_copy(w1t[:], pt[:])
                pv = ptr.tile([128, 1], F32, tag="pv")
                nc.tensor.matmul(
                    pv[:], w1t[:], a1rc_t[:, hd : hd + 1], start=True, stop=True
                )
                nc.vector.tensor_copy(W1ar_t[:, hd : hd + 1], pv[:])
            nc.vector.tensor_copy(W1arb[:], W1ar_t[:])
            pt2 = ptr.tile([128, 128], F32, tag="pt")
            nc.tensor.transpose(pt2[0:C, :], W2cat[:, 0:C], id_t[:])
            w2t = sp.tile([128, 128], F32, tag="w1t")
            nc.vector.tensor_copy(w2t[0:C, :], pt2[0:C, :])
            pv2 = ptr.tile([128, 1], F32, tag="pv")
            nc.tensor.matmul(
                pv2[:], w2t[0:C, :], a2rc_t[0:C, :], start=True, stop=True
            )
            nc.vector.tensor_copy(W2cat[:, C : C + 1], pv2[:])

        # ---------------- dense layer 1 (local slice only) ----------------
        h_nm = pp.tile([128, NWIN, OW1], F32)
        g1_nm = pp.tile([128, NWIN, H], F32)
        with (
            tc.tile_pool(name="xtp" + s, bufs=3) as xtp,
            tc.tile_pool(name="ph" + s, bufs=2, space="PSUM") as php,
            tc.tile_pool(name="psr" + s, bufs=2, space="PSUM") as psrp,
        ):
            for b in range(NWIN):
                xt = xtp.tile([128, 128], BF16)
                nc.sync.dma_start(
                    xt[:], T.bfblob[:, _XO + b * 128 : _XO + (b + 1) * 128]
                )
                ph = php.tile([128, OW1], F32)
                nc.tensor.matmul(ph[:], xt[:], W1_t[:], start=True, stop=True)
                psr = psrp.tile([128, H], F32)
                nc.tensor.matmul(psr[:], xt[:], W1arb[:], start=True, stop=True)
                nc.vector.tensor_copy(h_nm[:, b, :], ph[:])
                nc.scalar.activation(g1_nm[:, b, :], psr[:], EXP)

        # ---------------- g1 table slice + AllGather ----------------
        with tc.tile_pool(name="stage1" + s, bufs=1) as stp:
            st = stp.tile([128, NWIN, 128], BF16, tag="stg1")
            nc.vector.memset(st[:], 0.0)
            for b in range(NWIN):
                nc.vector.tensor_scalar(
                    st[:, b, 0:H], g1_nm[:, b, :], mask_t[:, b : b + 1], None, MULT
                )
            nc.sync.dma_start(
                T.g1_sl.ap().rearrange("(b p) c -> p b c", p=128), st[:]
            )
        nc.gpsimd.collective_compute(
            "AllGather", BYPASS, groups,
            ins=[T.g1_sl[:].opt()], outs=[T.g1_tab[:].opt()],
        )

        # ---------------- z1 ----------------
        u1_nm = pp.tile([128, NWIN, H], F32)

        def zflush1(w, po):
            zc = sp.tile([128, H], F32, tag="zc")
            nc.vector.tensor_scalar(zc[:], po[:, 0:H], EPS, None, MAX)
            nc.vector.reciprocal(u1_nm[:, w, :], zc[:])

        _spmm(nc, tc, zB, CH_Z, zidx_t, 0, rel_t, 0, T.g1_tab, 128, 8, 8,
              iof_t, "z1" + s, zflush1, bufs=2)

        # ---------------- hh1 table slice + AllGather ----------------
        with tc.tile_pool(name="hhp" + s, bufs=3) as hhp:
            for b in range(NWIN):
                hh = hhp.tile([128, OW1], BF16)
                for hd in range(H):
                    nc.vector.tensor_scalar(
                        hh[:, hd * F : (hd + 1) * F],
                        h_nm[:, b, hd * F : (hd + 1) * F],
                        u1_nm[:, b, hd : hd + 1],
                        None,
                        MULT,
                    )
                nc.sync.dma_start(
                    T.hh1_sl.ap().rearrange("(b p) c -> p b c", p=128)[:, b, :],
                    hh[:],
                )
        nc.gpsimd.collective_compute(
            "AllGather", BYPASS, groups,
            ins=[T.hh1_sl[:].opt()], outs=[T.hh1_tab[:].opt()],
        )

        # ---------------- agg1 (+ ELU + head mean + transpose) ----------------
        h1T_sb = pp.tile([128, SLICE], F32)
        with (
            tc.tile_pool(name="ptw" + s, bufs=2, space="PSUM") as ptw,
            tc.tile_pool(name="flush" + s, bufs=2) as flp,
        ):
            def flush1(w, po):
                o_t = flp.tile([128, OW1], F32, tag="o")
                for hd in range(H):
                    nc.vector.tensor_scalar(
                        o_t[:, hd * F : (hd + 1) * F],
                        po[:, hd * F : (hd + 1) * F],
                        g1_nm[:, w, hd : hd + 1],
                        None, MULT,
                    )
                # elu(x) = relu(x) + exp(min(x,0)) - 1 ; h1 = mean_heads
                neg = flp.tile([128, OW1], F32, tag="neg")
                nc.vector.tensor_scalar(neg[:], o_t[:], 0.0, None, MIN)
                ex = flp.tile([128, OW1], F32, tag="ex")
                nc.scalar.activation(ex[:], neg[:], EXP)
                rl = flp.tile([128, OW1], F32, tag="rl")
                nc.vector.tensor_relu(rl[:], o_t[:])
                su = flp.tile([128, OW1], F32, tag="su")
                nc.vector.tensor_tensor(su[:], rl[:], ex[:], ADD)
                t01 = flp.tile([128, F], F32, tag="t01")
                nc.vector.tensor_tensor(t01[:], su[:, 0:F], su[:, F : 2 * F], ADD)
                t23 = flp.tile([128, F], F32, tag="t23")
                nc.vector.tensor_tensor(
                    t23[:], su[:, 2 * F : 3 * F], su[:, 3 * F :], ADD
                )
                h1_t = flp.tile([128, F], F32, tag="h1")
                nc.vector.tensor_tensor(h1_t[:], t01[:], t23[:], ADD)
                nc.vector.tensor_scalar(h1_t[:], h1_t[:], 0.25, -1.0, MULT, ADD)
                ptt = ptw.tile([128, 128], F32, tag="ptt")
                nc.tensor.transpose(ptt[:], h1_t[:], id_t[:])
                nc.vector.tensor_copy(h1T_sb[:, w * 128 : (w + 1) * 128], ptt[:])

            _spmm(nc, tc, bB, CH_A1, bidx_t, 0, rel_t, ZT_z, T.hh1_tab, OW1,
                  OW1, OW1, iof_t, "a1" + s, flush1, bufs=3)

        # ---------------- dense layer 2 (local slice) ----------------
        h2_nm = pp.tile([128, NWIN, C], F32)
        g2_nm = pp.tile([128, NWIN, 1], F32)
        with tc.tile_pool(name="ph2" + s, bufs=2, space="PSUM") as ph2p:
            for b in range(NWIN):
                ph2 = ph2p.tile([128, C + 1], F32)
                nc.tensor.matmul(
                    ph2[:], h1T_sb[:, b * 128 : (b + 1) * 128], W2cat[:],
                    start=True, stop=True,
                )
                nc.vector.tensor_copy(h2_nm[:, b, :], ph2[:, 0:C])
                nc.scalar.activation(g2_nm[:, b, :], ph2[:, C : C + 1], EXP)

        # ---------------- g2 table slice + AllGather ----------------
        with tc.tile_pool(name="stage2" + s, bufs=1) as stp:
            st = stp.tile([128, NWIN, 128], BF16, tag="stg2")
            nc.vector.memset(st[:], 0.0)
            for b in range(NWIN):
                nc.vector.tensor_scalar(
                    st[:, b, 0:1], g2_nm[:, b, :], mask_t[:, b : b + 1], None, MULT
                )
            nc.sync.dma_start(
                T.g2_sl.ap().rearrange("(b p) c -> p b c", p=128), st[:]
            )
        nc.gpsimd.collective_compute(
            "AllGather", BYPASS, groups,
            ins=[T.g2_sl[:].opt()], outs=[T.g2_tab[:].opt()],
        )

        # ---------------- z2 ----------------
        u2_nm = pp.tile([128, NWIN, 1], F32)

        def zflush2(w, po):
            zc = sp.tile([128, 1], F32, tag="zc2")
            nc.vector.tensor_scalar(zc[:], po[:, 0:1], EPS, None, MAX)
            nc.vector.reciprocal(u2_nm[:, w, :], zc[:])

        _spmm(nc, tc, zB, CH_Z, zidx_t, 0, rel_t, 0, T.g2_tab, 128, 8, 8,
              iof_t, "z2" + s, zflush2, bufs=3)

        # ---------------- hh2 table slice + AllGather ----------------
        with tc.tile_pool(name="stage3" + s, bufs=1) as stp:
            st = stp.tile([128, NWIN, 128], BF16, tag="stg3")
            nc.vector.memset(st[:], 0.0)
            for b in range(NWIN):
                nc.vector.tensor_scalar(
                    st[:, b, 0:C], h2_nm[:, b, :], u2_nm[:, b, 0:1], None, MULT
                )
            nc.sync.dma_start(
                T.hh2_sl.ap().rearrange("(b p) c -> p b c", p=128), st[:]
            )
        nc.gpsimd.collective_compute(
            "AllGather", BYPASS, groups,
            ins=[T.hh2_sl[:].opt()], outs=[T.hh2_tab[:].opt()],
        )

        # ---------------- agg2 -> output ----------------
        with tc.tile_pool(name="fl2" + s, bufs=2) as flp:

            def flush2(w, po):
                o2 = flp.tile([128, C], BF16, tag="o2")
                nc.vector.tensor_scalar(
                    o2[:], po[:, 0:C], g2_nm[:, w, 0:1], None, MULT
                )
                nc.sync.dma_start(
                    T.out_d[w * 128 : (w + 1) * 128, :], o2[:]
                )

            _spmm(nc, tc, bB, CH_A2, bidx_t, 0, rel_t, ZT_z, T.hh2_tab, 128,
                  C, C, iof_t, "a2" + s, flush2, bufs=3)


def _build_program(zB, bB, reps=1):
    nc = bacc.Bacc("TRN2", target_bir_lowering=False, debug=False, num_devices=P)
    T = _declare(nc, sum(zB), sum(bB))
    with tile.TileContext(nc) as tc:
        for r in range(reps):
            _emit(nc, tc, T, zB, bB, s=str(r) if reps > 1 else "")
            if reps > 1:
                with tc.tile_critical():
                    nc.all_core_barrier()
    nc.compile()
    return nc


def _host_inputs(x, W1, a1, W2, a2, per_core):
    import ml_dtypes

    BF = ml_dtypes.bfloat16
    xT = np.zeros((F, NPAD), np.float32)
    xT[:, :N] = np.ascontiguousarray(np.asarray(x, np.float32).T)
    a1 = np.asarray(a1, np.float32)
    a2 = np.asarray(a2, np.float32)
    a1rc = np.ascontiguousarray(a1[:, F : 2 * F].T)  # [128, H]
    a2rc = np.zeros((F, 1), np.float32)
    a2rc[0:C, 0] = a2[0, C : 2 * C]
    W1 = np.asarray(W1, np.float32)
    W2 = np.asarray(W2, np.float32)
    iota_f = np.tile(np.arange(128, dtype=np.float32), (128, 1))
    iota_p = np.arange(128, dtype=np.float32).reshape(128, 1)
    ids = np.arange(NPAD)
    in_maps = []
    for k in range(P):
        base = k * SLICE
        mask = (
            (ids[base : base + SLICE] < N)
            .astype(np.float32)
            .reshape(NWIN, 128)
            .T
        )
        zidx, zrel, bidx, brel = per_core[k]

        def idx128(w16):
            # [16, ZT*8] i16 -> [128, ZT] rows q*16+j = wrapped[j, chunk q]
            n8 = w16.shape[1]
            return np.ascontiguousarray(
                w16.reshape(16, 8, n8 // 8).transpose(1, 0, 2).reshape(128, n8 // 8)
            ).view(BF)

        bfb = np.concatenate(
            [
                xT[:, base : base + SLICE].astype(BF),
                W1.astype(BF), W2.astype(BF), a1rc.astype(BF),
                a2rc.astype(BF), np.ascontiguousarray(mask).astype(BF),
                iota_f.astype(BF), iota_p.astype(BF),
                np.asarray(zrel, BF), np.asarray(brel, BF),
                idx128(zidx), idx128(bidx),
            ],
            axis=1,
        )
        in_maps.append(dict(bfblob=bfb))
    return in_maps


def build(x, edge_index, W1, a1, W2, a2, reps=1):
    """Build program + per-core input maps. Returns (nc, in_maps)."""
    ei = np.asarray(edge_index)
    row = ei[0].astype(np.int64)
    col = ei[1].astype(np.int64)
    zB, bB, per_core = _build_edge_inputs(row, col)
    nc = _build_program(zB, bB, reps=reps)
    in_maps = _host_inputs(x, W1, a1, W2, a2, per_core)
    return nc, in_maps


def kernel(x, edge_index, W1, a1, W2, a2):
    nc, in_maps = build(x, edge_index, W1, a1, W2, a2)
    res = run_bass_kernel_spmd(nc, in_maps, list(range(P)))
    out = np.concatenate(
        [np.asarray(res.results[k]["out"], np.float32) for k in range(P)], axis=0
    )
    return out[:N]
